# revision 12
# baseline (speedup 1.0000x reference)
"""Ewald potential Bass kernels for TRN2 (8-core SPMD), v2.

K1 shards k-space (480 cols/core of padded 3840) over all 8192 atoms ->
k_pot re/im (fp32) and v_pot re/im (fp16). Host gathers, computes
akp=|k_pot| and fp16 splits. K2 shards atoms (1024/core): aw GEMM (3-term
fp16 split) -> softmax -> inverse transform via PE-transposed sm.

Phases come from a one-hot selection GEMM against host-precomputed
frac(coord*k) tables centered in [-0.5,0.5]: phase' = Tx+Ty+Tz in
[-1.5,1.5], range-reduced with a single ADD_RANGE_WRAP, cos via a second
wrap (+0.25). Sin activation with scale=2pi.

out[n,d] = sum_k sm[n,k] * (cos(2pi phi_i)*vpr[k,d] + sin(2pi phi_i)*vpi[k,d]) / Z[n]
with eik_i = exp(-2pi i phi_i) = cos - i sin.
"""
import sys
sys.path.insert(0, '/opt/trn_rl_repo')
import numpy as np
import ml_dtypes
import concourse.bass as bass
import concourse.tile as tile
import concourse.mybir as mybir
from concourse import bacc
from concourse.bass_utils import run_bass_kernel_spmd
from contextlib import ExitStack

F = mybir.ActivationFunctionType
DT = mybir.dt
ALU = mybir.AluOpType
AX = mybir.AxisListType

P = 128
N = 8192
D = 128
NK = 12              # grid: kx in [0,12], ky/kz in [-12,12]
KPAD = 3840          # 3796 padded to 30*128
KSH = KPAD // 8      # 480 k-cols per core in K1
NSH = N // 8         # 1024 atoms per core in K2
NCH = N // P         # 64 atom chunks in K1
KCH = KPAD // P      # 30 k chunks in K2
AWK = 4096           # aw/sm width per n-chunk (2 halves of 2048)
NC2 = NSH // P       # 8 atom chunks in K2
NROW = 63            # 13 x-rows + 25 y-rows + 25 z-rows
TWOPI = float(2 * np.pi)

bf16 = ml_dtypes.bfloat16
f16 = np.float16


def _frac_tables(rfrac):
    """[63, n] fp64 tables: frac(coord*u) centered to [-0.5, 0.5]."""
    n = rfrac.shape[0]
    t = np.zeros((NROW, n), dtype=np.float64)
    r64 = rfrac.astype(np.float64)
    for u in range(NK + 1):                      # x rows: u = 0..12
        v = r64[:, 0] * u
        t[u] = v - np.round(v)
    for i, u in enumerate(range(-NK, NK + 1)):   # y rows
        v = r64[:, 1] * u
        t[13 + i] = v - np.round(v)
    for i, u in enumerate(range(-NK, NK + 1)):   # z rows
        v = r64[:, 2] * u
        t[38 + i] = v - np.round(v)
    return t


def _select_mat(kmat):
    """[63, KPAD] fp16 one-hot selection for k rows (padded cols zero)."""
    K = kmat.shape[0]
    s = np.zeros((NROW, KPAD), dtype=np.float32)
    j = np.arange(K)
    s[kmat[:, 0], j] = 1.0
    s[13 + kmat[:, 1] + NK, j] = 1.0
    s[38 + kmat[:, 2] + NK, j] = 1.0
    return s.astype(f16)


def split16(x):
    """fp16 2-way split: x ~ hi + lo to ~2^-22 rel."""
    x = np.asarray(x, dtype=np.float32)
    hi = x.astype(f16)
    lo = (x - hi.astype(np.float32)).astype(f16)
    return hi, lo


def host_prep(q_vector, k_vector, v_vector, positions, cell, k_fwd, k_inv):
    L = float(np.asarray(cell).reshape(3, 3)[0, 0])
    rfrac = np.asarray(positions, dtype=np.float64) / L
    t64 = _frac_tables(rfrac)                     # [63, N]
    th = t64.astype(f16)
    tl = (t64 - th.astype(np.float64)).astype(f16)
    sf = _select_mat(np.asarray(k_fwd))           # [63, KPAD]
    si = _select_mat(np.asarray(k_inv))
    kvh, kvl = split16(k_vector)                  # [N, D]
    vvh = np.asarray(v_vector, dtype=np.float32).astype(f16)
    qh, ql = split16(np.abs(np.asarray(q_vector, dtype=np.float32)).T)  # [D, N]
    return th, tl, sf, si, kvh, kvl, vvh, qh, ql


def chunk_major(x):
    """[N, D] -> [P, NCH*D]: partition=n%P? No: chunk c rows c*P..(c+1)*P
    land at [:, c*D:(c+1)*D]."""
    n, d = x.shape
    c = n // P
    return np.ascontiguousarray(
        x.reshape(c, P, d).transpose(1, 0, 2).reshape(P, c * d))


# ---------------------------------------------------------------- kernel 1
def build_k1():
    nc = bacc.Bacc("TRN2", target_bir_lowering=False, debug=False)
    th_d = nc.dram_tensor("th", [NROW, N], DT.float16, kind="ExternalInput").ap()
    tl_d = nc.dram_tensor("tl", [NROW, N], DT.float16, kind="ExternalInput").ap()
    sf_d = nc.dram_tensor("sf", [NROW, KSH], DT.float16, kind="ExternalInput").ap()
    kvh_d = nc.dram_tensor("kvh", [P, NCH * D], DT.float16, kind="ExternalInput").ap()
    vvh_d = nc.dram_tensor("vvh", [P, NCH * D], DT.float16, kind="ExternalInput").ap()
    kre_d = nc.dram_tensor("kre", [P, KSH], DT.float32, kind="ExternalOutput").ap()
    kim_d = nc.dram_tensor("kim", [P, KSH], DT.float32, kind="ExternalOutput").ap()
    vre_d = nc.dram_tensor("vre", [P, KSH], DT.float16, kind="ExternalOutput").ap()
    vim_d = nc.dram_tensor("vim", [P, KSH], DT.float16, kind="ExternalOutput").ap()

    with ExitStack() as ctx:
        tc = ctx.enter_context(tile.TileContext(nc))
        cpool = ctx.enter_context(tc.tile_pool(name="const", bufs=1))
        wpool = ctx.enter_context(tc.tile_pool(name="work", bufs=3))
        php = ctx.enter_context(tc.tile_pool(name="ph", bufs=4, space="PSUM"))
        accp = ctx.enter_context(tc.tile_pool(name="acc", bufs=1, space="PSUM"))

        th = cpool.tile([NROW, N], DT.float16)
        tlo = cpool.tile([NROW, N], DT.float16)
        sf = cpool.tile([NROW, KSH], DT.float16)
        kvh = cpool.tile([P, NCH * D], DT.float16)
        vvh = cpool.tile([P, NCH * D], DT.float16)
        nc.sync.dma_start(sf[:], sf_d)
        nc.sync.dma_start(th[:], th_d)
        nc.sync.dma_start(tlo[:], tl_d)
        nc.sync.dma_start(kvh[:], kvh_d)
        nc.sync.dma_start(vvh[:], vvh_d)

        kre = accp.tile([P, 512], DT.float32)
        kim = accp.tile([P, 512], DT.float32)
        vre = accp.tile([P, 512], DT.float32)
        vim = accp.tile([P, 512], DT.float32)

        phs = {}

        def emit_ph(c):
            if c >= NCH:
                return
            t = php.tile([P, 512], DT.float32, tag="ph")
            nc.tensor.matmul(t[:, :KSH], th[:, c * P:(c + 1) * P], sf[:],
                             start=True, stop=False)
            nc.tensor.matmul(t[:, :KSH], tlo[:, c * P:(c + 1) * P], sf[:],
                             start=False, stop=True)
            phs[c] = t

        def emit_acc(p, sinf, cosf):
            # cos-consumers first: kre/vre, then kim/vim
            for h in range(2):
                c = 2 * p + h
                sl = slice(h * 512, h * 512 + KSH)
                dsl = slice(c * D, (c + 1) * D)
                nc.tensor.matmul(kre[:, :KSH], kvh[:, dsl], cosf[:, sl],
                                 start=(c == 0), stop=(c == NCH - 1))
                nc.tensor.matmul(vre[:, :KSH], vvh[:, dsl], cosf[:, sl],
                                 start=(c == 0), stop=(c == NCH - 1))
            for h in range(2):
                c = 2 * p + h
                sl = slice(h * 512, h * 512 + KSH)
                dsl = slice(c * D, (c + 1) * D)
                nc.tensor.matmul(kim[:, :KSH], kvh[:, dsl], sinf[:, sl],
                                 start=(c == 0), stop=(c == NCH - 1))
                nc.tensor.matmul(vim[:, :KSH], vvh[:, dsl], sinf[:, sl],
                                 start=(c == 0), stop=(c == NCH - 1))

        for c in range(4):
            emit_ph(c)
        prev = None          # (sinf, cosf) of pair p-1
        for p in range(NCH // 2):
            a, b = 2 * p, 2 * p + 1
            r = wpool.tile([P, 1024], DT.float32, tag="r")
            w2 = wpool.tile([P, 1024], DT.float32, tag="w2")
            nc.vector.add_range_wrap(r[:, 0:512], phs[a][:], 0.0, 0.5, 1.0)
            nc.vector.add_range_wrap(w2[:, 0:512], r[:, 0:512], 0.25, 0.5, 1.0)
            nc.vector.add_range_wrap(r[:, 512:1024], phs[b][:], 0.0, 0.5, 1.0)
            nc.vector.add_range_wrap(w2[:, 512:1024], r[:, 512:1024], 0.25, 0.5,
                                     1.0)
            del phs[a], phs[b]
            emit_ph(2 * p + 4)
            emit_ph(2 * p + 5)
            sinf = wpool.tile([P, 1024], DT.float16, tag="sinf")
            cosf = wpool.tile([P, 1024], DT.float16, tag="cosf")
            nc.scalar.activation(cosf[:], w2[:], F.Sin, scale=TWOPI)
            nc.scalar.activation(sinf[:], r[:], F.Sin, scale=TWOPI)
            if prev is not None:
                emit_acc(p - 1, *prev)
            prev = (sinf, cosf)
        emit_acc(NCH // 2 - 1, *prev)

        kre_s = wpool.tile([P, KSH], DT.float32, tag="kre_s")
        kim_s = wpool.tile([P, KSH], DT.float32, tag="kim_s")
        vre_s = wpool.tile([P, KSH], DT.float16, tag="vre_s")
        vim_s = wpool.tile([P, KSH], DT.float16, tag="vim_s")
        nc.vector.tensor_copy(kre_s[:], kre[:, :KSH])
        nc.vector.tensor_copy(kim_s[:], kim[:, :KSH])
        nc.vector.tensor_copy(vre_s[:], vre[:, :KSH])
        nc.vector.tensor_copy(vim_s[:], vim[:, :KSH])
        nc.sync.dma_start(kre_d, kre_s[:])
        nc.sync.dma_start(kim_d, kim_s[:])
        nc.sync.dma_start(vre_d, vre_s[:])
        nc.sync.dma_start(vim_d, vim_s[:])

    nc.compile()
    return nc


# ---------------------------------------------------------------- kernel 2
def build_k2():
    nc = bacc.Bacc("TRN2", target_bir_lowering=False, debug=False)
    t2_d = nc.dram_tensor("t2", [NROW, NSH], DT.float16, kind="ExternalInput").ap()
    si_d = nc.dram_tensor("si", [NROW, KPAD], DT.float16, kind="ExternalInput").ap()
    qh_d = nc.dram_tensor("qh", [P, NSH], DT.float16, kind="ExternalInput").ap()
    ah_d = nc.dram_tensor("ah", [P, AWK], DT.float16, kind="ExternalInput").ap()
    al_d = nc.dram_tensor("al", [P, AWK], DT.float16, kind="ExternalInput").ap()
    vprT_d = nc.dram_tensor("vprT", [P, KCH * D], DT.bfloat16,
                            kind="ExternalInput").ap()
    vpiT_d = nc.dram_tensor("vpiT", [P, KCH * D], DT.bfloat16,
                            kind="ExternalInput").ap()
    ident_d = nc.dram_tensor("ident", [P, P], DT.bfloat16, kind="ExternalInput").ap()
    outT_d = nc.dram_tensor("outT", [P, NSH], DT.float32, kind="ExternalOutput").ap()
    zs_d = nc.dram_tensor("zs", [P, 3 * NC2], DT.float32, kind="ExternalOutput").ap()

    with ExitStack() as ctx:
        tc = ctx.enter_context(tile.TileContext(nc))
        cpool = ctx.enter_context(tc.tile_pool(name="const", bufs=1))
        smpool = ctx.enter_context(tc.tile_pool(name="sm", bufs=1))
        wpool = ctx.enter_context(tc.tile_pool(name="work", bufs=3))
        zpool = ctx.enter_context(tc.tile_pool(name="z", bufs=1))

        t2 = cpool.tile([NROW, NSH], DT.float16)
        si = cpool.tile([NROW, KPAD], DT.float16)
        qh = cpool.tile([P, NSH], DT.float16)
        ah = cpool.tile([P, AWK], DT.float16)
        al = cpool.tile([P, AWK], DT.float16)
        vprT = cpool.tile([P, KCH * D], DT.bfloat16)
        vpiT = cpool.tile([P, KCH * D], DT.bfloat16)
        ident = cpool.tile([P, P], DT.bfloat16)
        nc.sync.dma_start(qh[:], qh_d)
        nc.sync.dma_start(ah[:], ah_d)
        nc.sync.dma_start(al[:], al_d)
        nc.sync.dma_start(si[:], si_d)
        nc.sync.dma_start(t2[:], t2_d)
        nc.sync.dma_start(vprT[:], vprT_d)
        nc.sync.dma_start(vpiT[:], vpiT_d)
        nc.sync.dma_start(ident[:], ident_d)

        sm = smpool.tile([P, NC2 * 4096], DT.bfloat16)
        zacc = zpool.tile([P, 3 * NC2], DT.float32)

        # ---- pass 1: aw (2-term fp16) -> softmax; quarters q0/q1 staged to
        # SBUF so their PSUM banks free early and the PE never stalls.
        with tc.tile_pool(name="awps", bufs=1, space="PSUM") as awps:
            for c8 in range(NC2):
                nsl = slice(c8 * P, (c8 + 1) * P)
                awq = []
                mxs = []
                stage = wpool.tile([P, 2048], DT.float32, tag="stage")
                for q in range(4):
                    aw = awps.tile([P, 1024], DT.float32, tag=f"aw{q}")
                    for j in range(2):
                        ksl = slice(q * 1024 + j * 512, q * 1024 + (j + 1) * 512)
                        osl = slice(j * 512, (j + 1) * 512)
                        nc.tensor.matmul(aw[:, osl], qh[:, nsl], ah[:, ksl],
                                         start=True, stop=False)
                        nc.tensor.matmul(aw[:, osl], qh[:, nsl], al[:, ksl],
                                         start=False, stop=True)
                    mx = wpool.tile([P, 1], DT.float32, tag=f"mx{q}")
                    nc.vector.reduce_max(mx[:], aw[:], axis=AX.X)
                    if q < 2:
                        nc.vector.tensor_copy(
                            stage[:, q * 1024:(q + 1) * 1024], aw[:])
                    awq.append(aw)
                    mxs.append(mx)
                mxa = wpool.tile([P, 1], DT.float32, tag="mxa")
                mxb = wpool.tile([P, 1], DT.float32, tag="mxb")
                nc.vector.tensor_tensor(mxa[:], mxs[0][:], mxs[1][:], ALU.max)
                nc.vector.tensor_tensor(mxb[:], mxs[2][:], mxs[3][:], ALU.max)
                mxc = wpool.tile([P, 1], DT.float32, tag="mxc")
                nc.vector.tensor_tensor(mxc[:], mxa[:], mxb[:], ALU.max)
                negmx = wpool.tile([P, 1], DT.float32, tag="negmx")
                nc.vector.tensor_scalar_mul(negmx[:], mxc[:], -1.0)
                nc.scalar.activation(
                    sm[:, c8 * AWK: c8 * AWK + 2048], stage[:], F.Exp,
                    bias=negmx[:], accum_out=zacc[:, 3 * c8: 3 * c8 + 1])
                for q in (2, 3):
                    nc.scalar.activation(
                        sm[:, c8 * AWK + q * 1024: c8 * AWK + (q + 1) * 1024],
                        awq[q][:], F.Exp, bias=negmx[:],
                        accum_out=zacc[:, 3 * c8 + q - 1: 3 * c8 + q])

        # ---- pass 2: phases -> sin/cos -> PE-transpose sm -> inverse GEMM
        with (tc.tile_pool(name="php", bufs=4, space="PSUM") as php,
              tc.tile_pool(name="smtp", bufs=2, space="PSUM") as smtp,
              tc.tile_pool(name="ops", bufs=1, space="PSUM") as ops):
            outT = ops.tile([P, NSH], DT.float32)
            phs = {}

            def emit_ph(j):          # j = half index 0..59 (2 per k-chunk)
                if j >= 2 * KCH:
                    return
                t = php.tile([P, 512], DT.float32, tag="ph")
                nc.tensor.matmul(t[:], si[:, (j // 2) * P:(j // 2 + 1) * P],
                                 t2[:, (j % 2) * 512:(j % 2 + 1) * 512],
                                 start=True, stop=True)
                phs[j] = t

            def emit_inv(kc, smC, smS):
                dsl = slice(kc * D, (kc + 1) * D)
                for h in range(2):
                    sl = slice(h * 512, (h + 1) * 512)
                    nc.tensor.matmul(outT[:, sl], vprT[:, dsl], smC[:, sl],
                                     start=(kc == 0), stop=False)
                    nc.tensor.matmul(outT[:, sl], vpiT[:, dsl], smS[:, sl],
                                     start=False, stop=(kc == KCH - 1))

            for j in range(4):
                emit_ph(j)
            prev = None
            for kc in range(KCH):
                r = wpool.tile([P, 1024], DT.float32, tag="r")
                w2 = wpool.tile([P, 1024], DT.float32, tag="w2")
                for h in range(2):
                    j = 2 * kc + h
                    sl = slice(h * 512, (h + 1) * 512)
                    nc.vector.add_range_wrap(r[:, sl], phs[j][:], 0.0, 0.5, 1.0)
                    nc.vector.add_range_wrap(w2[:, sl], r[:, sl], 0.25, 0.5, 1.0)
                    del phs[j]
                emit_ph(2 * kc + 4)
                emit_ph(2 * kc + 5)
                smT = smtp.tile([P, 1024], DT.bfloat16, tag="smT")
                for c8 in range(NC2):
                    nc.tensor.transpose(
                        smT[:, c8 * P:(c8 + 1) * P],
                        sm[:, c8 * AWK + kc * P: c8 * AWK + (kc + 1) * P],
                        ident[:])
                sint = wpool.tile([P, 1024], DT.bfloat16, tag="sint")
                cost = wpool.tile([P, 1024], DT.bfloat16, tag="cost")
                nc.scalar.activation(cost[:], w2[:], F.Sin, scale=TWOPI)
                nc.scalar.activation(sint[:], r[:], F.Sin, scale=TWOPI)
                smC = wpool.tile([P, 1024], DT.bfloat16, tag="smC")
                smS = wpool.tile([P, 1024], DT.bfloat16, tag="smS")
                nc.vector.tensor_mul(smC[:], smT[:], cost[:])
                nc.vector.tensor_mul(smS[:], smT[:], sint[:])
                if prev is not None:
                    emit_inv(kc - 1, *prev)
                prev = (smC, smS)
            emit_inv(KCH - 1, *prev)

            res = wpool.tile([P, NSH], DT.float32, tag="res")
            nc.vector.tensor_copy(res[:], outT[:])
            nc.sync.dma_start(outT_d, res[:])
            nc.sync.dma_start(zs_d, zacc[:])

    nc.compile()
    return nc


# ---------------------------------------------------------------- profiling
def enable_ntff_profiling():
    """Provide the antenv.axon_hooks module run_bass_kernel_spmd needs for
    trace=True under axon, backed by trn_boot's ctypes NTFF hook."""
    import types
    if "antenv.axon_hooks" in sys.modules:
        return True
    sys.path.insert(0, "/root/.axon_site")
    try:
        from trn_agent_boot.trn_boot import _ntff_profile_via_ctypes
        hook = _ntff_profile_via_ctypes("/opt/axon/libaxon_pjrt.so")
    except Exception as e:
        print(f"ntff hook unavailable: {e}")
        return False
    if hook is None:
        print("ntff hook: .so lacks axon_start_nrt_profile")
        return False
    mod = types.ModuleType("antenv.axon_hooks")
    mod._hook = hook
    mod.get_axon_ntff_profile_hook = lambda: mod._hook
    mod.set_axon_ntff_profile_hook = lambda h: setattr(mod, "_hook", h)
    sys.modules["antenv.axon_hooks"] = mod
    # upload_artifacts copies the NEFF dir to a remote bucket -- hangs in
    # this container; keep artifacts local instead.
    import concourse.bass_utils as bu
    bu.upload_artifacts = lambda tmpdir: tmpdir
    return True


# ---------------------------------------------------------------- runner
_NC1 = None
_NC2 = None


def run_ewald(q_vector, k_vector, v_vector, positions, cell, batch, k_fwd,
              k_inv, trace=False):
    global _NC1, _NC2
    if trace:
        trace = enable_ntff_profiling()
    th, tl, sf, si, kvh, kvl, vvh, qh, ql = host_prep(
        q_vector, k_vector, v_vector, positions, cell, k_fwd, k_inv)

    kvh_c = chunk_major(kvh)
    vvh_c = chunk_major(vvh)

    if _NC1 is None:
        _NC1 = build_k1()
    in1 = [{"th": th, "tl": tl,
            "sf": np.ascontiguousarray(sf[:, c * KSH:(c + 1) * KSH]),
            "kvh": kvh_c, "vvh": vvh_c} for c in range(8)]
    r1 = run_bass_kernel_spmd(_NC1, in1, list(range(8)), trace=trace)

    K = k_fwd.shape[0]
    kre = np.concatenate([r1.results[c]["kre"] for c in range(8)], axis=1)
    kim = np.concatenate([r1.results[c]["kim"] for c in range(8)], axis=1)
    vre = np.concatenate(
        [r1.results[c]["vre"].astype(np.float32) for c in range(8)], axis=1)
    vim = np.concatenate(
        [r1.results[c]["vim"].astype(np.float32) for c in range(8)], axis=1)
    akp = np.zeros((D, AWK), dtype=np.float32)
    akp[:, :KPAD] = np.hypot(kre, kim)
    akp[:, K:] = 0.0
    ah, al = split16(akp)
    vprT = chunk_major(np.ascontiguousarray(vre.T).astype(bf16))  # [P, KCH*D]
    vpiT = chunk_major(np.ascontiguousarray(vim.T).astype(bf16))
    ident = np.eye(P, dtype=np.float32).astype(bf16)

    if _NC2 is None:
        _NC2 = build_k2()
    in2 = [{"t2": np.ascontiguousarray(th[:, c * NSH:(c + 1) * NSH]),
            "si": si,
            "qh": np.ascontiguousarray(qh[:, c * NSH:(c + 1) * NSH]),
            "ah": ah, "al": al, "vprT": vprT, "vpiT": vpiT, "ident": ident}
           for c in range(8)]
    r2 = run_bass_kernel_spmd(_NC2, in2, list(range(8)), trace=trace)

    outs = []
    for c in range(8):
        oT = r2.results[c]["outT"]                    # [128 d, 1024 n]
        zs = r2.results[c]["zs"]                      # [128, 16]
        z = (zs[:, 0::3] + zs[:, 1::3] + zs[:, 2::3]).T.reshape(-1)
        outs.append((oT.T / z[:, None]).astype(np.float32))
    out = np.concatenate(outs, axis=0)
    return out, (r1, r2)


# ---------------------------------------------------------------- entry point
def kernel(q_vector, k_vector, v_vector, positions, cell, batch, k_fwd, k_inv):
    """Full-input entry: shards across 8 NeuronCores internally."""
    out, _ = run_ewald(np.asarray(q_vector), np.asarray(k_vector),
                       np.asarray(v_vector), np.asarray(positions),
                       np.asarray(cell), np.asarray(batch),
                       np.asarray(k_fwd), np.asarray(k_inv))
    return out
